# revision 28
# baseline (speedup 1.0000x reference)
"""MinGRU cell on 8 TRN2 NeuronCores.

Math (per batch b):
    g = sigmoid(x @ Wg.T + bg)          # [L, D]
    c = tanh(x @ Wh.T + bh)             # [L, D]
    h_t = g_t * h_{t-1} + (1 - g_t) * c_t   (h_0 init = hidden)

Sharding: data-parallel over batch B=8 -> one batch per core, no collectives.

Device layout: everything is kept "D on partitions, L on free dim":
  - host feeds xT = x[b].T  [D, L]  (contiguous DMA loads)
  - matmuls compute outT tiles [e_block=128, token_chunk=512] with PSUM
    accumulation over the 8 k-blocks of D
  - ScalarE applies sigmoid/tanh with the per-partition bias fused
  - VectorE computes d1 = (g-1)*c, then tensor_tensor_scan gives
    h = g*h_prev - d1 = g*h_prev + (1-g)*c along the free (token) dim
  - output is written back as outT [D, L]; host transposes to [L, D]

Matmul dtype: float32r (full-rate fp32 on the PE for N>=256).
"""

import numpy as np

import concourse.bacc as bacc
import concourse.tile as tile
import concourse.mybir as mybir
from concourse import bass_utils

B = 8
L = 4096
D = 1024
P = 128
NCH = 512          # token chunk (one fp32 PSUM bank)
KD = D // P        # 8 contraction blocks
NE = D // P        # 8 output-dim blocks
NCHUNK = L // NCH  # 8 token chunks

F32 = mybir.dt.float32
MM_DT = mybir.dt.float32r  # full-rate fp32 matmul


def build_nc(mm_dt=None, w_dt=None):
    global MM_DT, W_DT
    MM_DT = mm_dt or mybir.dt.float32r
    W_DT = w_dt or MM_DT
    nc = bacc.Bacc("TRN2", target_bir_lowering=False, debug=False)

    xT = nc.dram_tensor("xT", [D, L], MM_DT, kind="ExternalInput").ap()
    WgT = nc.dram_tensor("WgT", [D, D], W_DT, kind="ExternalInput").ap()
    WhT = nc.dram_tensor("WhT", [D, D], W_DT, kind="ExternalInput").ap()
    bg = nc.dram_tensor("bg", [D], F32, kind="ExternalInput").ap()
    bh = nc.dram_tensor("bh", [D], F32, kind="ExternalInput").ap()
    hidden = nc.dram_tensor("hidden", [D], F32, kind="ExternalInput").ap()
    outT = nc.dram_tensor("outT", [D, L], F32, kind="ExternalOutput").ap()

    xT_r = xT.rearrange("(kd p) l -> p kd l", p=P)      # [128, 8, 4096]
    out_r = outT.rearrange("(e p) l -> p e l", p=P)     # [128, 8, 4096]
    wgT_r = WgT.rearrange("(kd p) e -> p kd e", p=P)    # [128, 8, 1024]
    whT_r = WhT.rearrange("(kd p) e -> p kd e", p=P)
    bg_r = bg.rearrange("(e p) -> p e", p=P)            # [128, 8]
    bh_r = bh.rearrange("(e p) -> p e", p=P)
    h0_r = hidden.rearrange("(e p) -> p e", p=P)

    ACT = mybir.ActivationFunctionType
    ALU = mybir.AluOpType

    with tile.TileContext(nc) as tc:
        with (
            tc.tile_pool(name="const", bufs=1) as const,
            tc.tile_pool(name="xin", bufs=2) as xpool,
            tc.tile_pool(name="gc", bufs=3) as gc,
            tc.tile_pool(name="hout", bufs=2) as hpool,
            tc.tile_pool(name="psum", bufs=4, space="PSUM") as pp,
        ):
            # First x chunk + Wg weights are the startup critical path; they
            # go on the ACT HWDGE ring.  Everything else (wh, outputs) rides
            # the Sync ring, and wh is paced behind phase-1 matmuls so the
            # packet-round-robin SDMA engines don't dilute the critical
            # stream.
            xin0 = xpool.tile([P, KD, NCH], MM_DT, tag="xin")
            nc.scalar.dma_start(out=xin0, in_=xT_r[:, :, 0:NCH])
            wg_sb = []
            wg_dma = []
            for kd in range(KD):
                wgt = const.tile([P, D], W_DT, tag=f"wg{kd}", name=f"wg_sb{kd}")
                wg_dma.append(nc.scalar.dma_start(out=wgt, in_=wgT_r[:, kd, :]))
                wg_sb.append(wgt)

            bg_sb = const.tile([P, NE], F32)
            bh_sb = const.tile([P, NE], F32)
            h0_sb = const.tile([P, NE], F32)
            nc.sync.dma_start(out=bg_sb, in_=bg_r)
            nc.sync.dma_start(out=bh_sb, in_=bh_r)
            nc.sync.dma_start(out=h0_sb, in_=h0_r)

            # ---- chunk 0, phase 1: kd-outer waves over 4 concurrent PSUM
            # banks; each wg[kd] DMA unblocks a whole wave level on arrival.
            gt0 = [None] * NE
            wave0_kd_mm = {}
            phase1_mid_mm = None
            for wave in range(2):
                es = list(range(wave * 4, wave * 4 + 4))
                pgs = {
                    e: pp.tile([P, NCH], F32, tag="pg", bufs=6, name=f"pg0_{e}")
                    for e in es
                }
                for kd in range(KD):
                    for e in es:
                        mm = nc.tensor.matmul(
                            pgs[e],
                            lhsT=wg_sb[kd][:, e * P : (e + 1) * P],
                            rhs=xin0[:, kd, :],
                            start=(kd == 0),
                            stop=(kd == KD - 1),
                        )
                    if wave == 0:
                        wave0_kd_mm[kd] = mm
                for e in es:
                    g = gc.tile([P, NCH], F32, tag=f"g{e}", name=f"g0_{e}")
                    nc.scalar.activation(
                        out=g, in_=pgs[e], func=ACT.Sigmoid,
                        bias=bg_sb[:, e : e + 1],
                    )
                    gt0[e] = g
                if wave == 0:
                    phase1_mid_mm = mm

            # Wh weights stream while phase 1 runs: wh[kd] waits on the
            # wave-0 matmul that consumed wg[kd].
            wh_sb = []
            for kd in range(KD):
                wht = const.tile([P, D], W_DT, tag=f"wh{kd}", name=f"wh_sb{kd}")
                dma = nc.sync.dma_start(out=wht, in_=whT_r[:, kd, :])
                tile.add_dep_helper(
                    dma.ins, wave0_kd_mm[kd].ins, sync=True, reason="pace wh behind wg"
                )
                wh_sb.append(wht)

            prev_h = [None] * NE

            def c_unit(n, e, gtile, xin, t0=0, tn=NCH):
                """c projection + pointwise + scan + store for tokens
                [t0, tn) of chunk n, output block e.  gtile holds the full
                chunk's g; the sub-range is sliced out of it."""
                w = tn - t0
                lsl = slice(n * NCH + t0, n * NCH + tn)
                tsl = slice(t0, tn)
                esl = slice(e * P, (e + 1) * P)
                pc = pp.tile([P, w], F32, tag="pc", bufs=2, name=f"pc_{n}_{e}_{t0}")
                for kd in range(KD):
                    nc.tensor.matmul(
                        pc,
                        lhsT=wh_sb[kd][:, esl],
                        rhs=xin[:, kd, tsl],
                        start=(kd == 0),
                        stop=(kd == KD - 1),
                    )
                c = gc.tile([P, w], F32, tag="c", name=f"c_{n}_{e}_{t0}")
                nc.scalar.activation(
                    out=c, in_=pc, func=ACT.Tanh, bias=bh_sb[:, e : e + 1]
                )
                d1 = gc.tile([P, w], F32, tag="d1", name=f"d1_{n}_{e}_{t0}")
                nc.vector.scalar_tensor_tensor(
                    out=d1, in0=gtile[:, tsl], scalar=1.0, in1=c,
                    op0=ALU.subtract, op1=ALU.mult,
                )
                if n == 0 and t0 == 0:
                    init = h0_sb[:, e : e + 1]
                else:
                    pw = prev_h[e].shape[-1]
                    init = prev_h[e][:, pw - 1 : pw]
                h = hpool.tile([P, w], F32, tag=f"h{e}", name=f"h_{n}_{e}_{t0}")
                nc.vector.tensor_tensor_scan(
                    out=h, data0=gtile[:, tsl], data1=d1, initial=init,
                    op0=ALU.mult, op1=ALU.subtract,
                )
                prev_h[e] = h
                nc.sync.dma_start(out=out_r[:, e, lsl], in_=h)

            # ---- chunk 0, phase 2
            for e in range(NE):
                c_unit(0, e, gt0[e], xin0)

            # ---- chunks 1..7: interleaved per-e units
            for n in range(1, NCHUNK):
                lsl = slice(n * NCH, (n + 1) * NCH)
                xin = xpool.tile([P, KD, NCH], MM_DT, tag="xin", name=f"xin_{n}")
                dma = nc.scalar.dma_start(out=xin, in_=xT_r[:, :, lsl])
                if n == 1:
                    # keep xin1 from competing with the startup weight stream
                    tile.add_dep_helper(
                        dma.ins, phase1_mid_mm.ins, sync=True, reason="pace xin1"
                    )
                for e in range(NE):
                    esl = slice(e * P, (e + 1) * P)
                    pg = pp.tile([P, NCH], F32, tag="pg", bufs=6, name=f"pg_{n}_{e}")
                    for kd in range(KD):
                        nc.tensor.matmul(
                            pg,
                            lhsT=wg_sb[kd][:, esl],
                            rhs=xin[:, kd, :],
                            start=(kd == 0),
                            stop=(kd == KD - 1),
                        )
                    g = gc.tile([P, NCH], F32, tag=f"g{e}", name=f"g_{n}_{e}")
                    nc.scalar.activation(
                        out=g, in_=pg, func=ACT.Sigmoid, bias=bg_sb[:, e : e + 1]
                    )
                    if n == NCHUNK - 1 and e == NE - 1:
                        # Final unit: halve it so the very last
                        # tanh+scan+store tail is half as long.
                        c_unit(n, e, g, xin, 0, NCH // 2)
                        c_unit(n, e, g, xin, NCH // 2, NCH)
                    else:
                        c_unit(n, e, g, xin)

    nc.compile()
    return nc


_NC_CACHE = None


def _get_nc():
    global _NC_CACHE
    if _NC_CACHE is None:
        _NC_CACHE = build_nc()
    return _NC_CACHE


def kernel(x, hidden, Wg, bg, Wh, bh):
    x = np.ascontiguousarray(np.asarray(x, dtype=np.float32))
    hidden = np.ascontiguousarray(np.asarray(hidden, dtype=np.float32))
    Wg = np.asarray(Wg, dtype=np.float32)
    bg = np.ascontiguousarray(np.asarray(bg, dtype=np.float32))
    Wh = np.asarray(Wh, dtype=np.float32)
    bh = np.ascontiguousarray(np.asarray(bh, dtype=np.float32))

    nc = _get_nc()

    xT = np.ascontiguousarray(x.transpose(0, 2, 1))   # [B, D, L]
    WgT = np.ascontiguousarray(Wg.T)
    WhT = np.ascontiguousarray(Wh.T)

    in_maps = [
        {
            "xT": xT[b],
            "WgT": WgT,
            "WhT": WhT,
            "bg": bg,
            "bh": bh,
            "hidden": hidden[b],
        }
        for b in range(B)
    ]
    res = bass_utils.run_bass_kernel_spmd(nc, in_maps, core_ids=list(range(B)))
    out = np.stack([res.results[b]["outT"].T for b in range(B)])  # [B, L, D]
    return np.ascontiguousarray(out.astype(np.float32))


# revision 29
# speedup vs baseline: 1.1897x; 1.1897x over previous
"""MinGRU cell on 8 TRN2 NeuronCores.

Math (per batch b):
    g = sigmoid(x @ Wg.T + bg)          # [L, D]
    c = tanh(x @ Wh.T + bh)             # [L, D]
    h_t = g_t * h_{t-1} + (1 - g_t) * c_t   (h_0 init = hidden)

Sharding: data-parallel over batch B=8 -> one batch per core, no collectives.

Device layout: everything is kept "D on partitions, L on free dim":
  - host feeds xT = x[b].T  [D, L]  (contiguous DMA loads)
  - matmuls compute outT tiles [e_block=128, token_chunk=512] with PSUM
    accumulation over the 8 k-blocks of D
  - ScalarE applies sigmoid/tanh with the per-partition bias fused
  - VectorE computes d1 = (g-1)*c, then tensor_tensor_scan gives
    h = g*h_prev - d1 = g*h_prev + (1-g)*c along the free (token) dim
  - output is written back as outT [D, L]; host transposes to [L, D]

Matmul dtype: float32r (full-rate fp32 on the PE for N>=256).
"""

import numpy as np

import concourse.bacc as bacc
import concourse.tile as tile
import concourse.mybir as mybir
from concourse import bass_utils

B = 8
L = 4096
D = 1024
P = 128
NCH = 512          # token chunk (one fp32 PSUM bank)
KD = D // P        # 8 contraction blocks
NE = D // P        # 8 output-dim blocks
NCHUNK = L // NCH  # 8 token chunks

F32 = mybir.dt.float32
MM_DT = mybir.dt.float32r  # full-rate fp32 matmul


def build_nc(mm_dt=None, w_dt=None):
    global MM_DT, W_DT
    MM_DT = mm_dt or mybir.dt.float32r
    W_DT = w_dt or MM_DT
    nc = bacc.Bacc("TRN2", target_bir_lowering=False, debug=False)

    xT = nc.dram_tensor("xT", [D, L], MM_DT, kind="ExternalInput").ap()
    WgT = nc.dram_tensor("WgT", [D, D], W_DT, kind="ExternalInput").ap()
    WhT = nc.dram_tensor("WhT", [D, D], W_DT, kind="ExternalInput").ap()
    bg = nc.dram_tensor("bg", [D], F32, kind="ExternalInput").ap()
    bh = nc.dram_tensor("bh", [D], F32, kind="ExternalInput").ap()
    hidden = nc.dram_tensor("hidden", [D], F32, kind="ExternalInput").ap()
    outT = nc.dram_tensor("outT", [D, L], F32, kind="ExternalOutput").ap()

    xT_r = xT.rearrange("(kd p) l -> p kd l", p=P)      # [128, 8, 4096]
    out_r = outT.rearrange("(e p) l -> p e l", p=P)     # [128, 8, 4096]
    wgT_r = WgT.rearrange("(kd p) e -> p kd e", p=P)    # [128, 8, 1024]
    whT_r = WhT.rearrange("(kd p) e -> p kd e", p=P)
    bg_r = bg.rearrange("(e p) -> p e", p=P)            # [128, 8]
    bh_r = bh.rearrange("(e p) -> p e", p=P)
    h0_r = hidden.rearrange("(e p) -> p e", p=P)

    ACT = mybir.ActivationFunctionType
    ALU = mybir.AluOpType

    with tile.TileContext(nc) as tc:
        with (
            tc.tile_pool(name="const", bufs=1) as const,
            tc.tile_pool(name="xin", bufs=2) as xpool,
            tc.tile_pool(name="gc", bufs=3) as gc,
            tc.tile_pool(name="hout", bufs=2) as hpool,
            tc.tile_pool(name="psum", bufs=4, space="PSUM") as pp,
        ):
            # First x chunk + Wg weights are the startup critical path; they
            # go on the ACT HWDGE ring.  Everything else (wh, outputs) rides
            # the Sync ring, and wh is paced behind phase-1 matmuls so the
            # packet-round-robin SDMA engines don't dilute the critical
            # stream.
            xin0 = xpool.tile([P, KD, NCH], MM_DT, tag="xin")
            nc.scalar.dma_start(out=xin0, in_=xT_r[:, :, 0:NCH])
            wg_sb = []
            wg_dma = []
            for kd in range(KD):
                wgt = const.tile([P, D], W_DT, tag=f"wg{kd}", name=f"wg_sb{kd}")
                wg_dma.append(nc.scalar.dma_start(out=wgt, in_=wgT_r[:, kd, :]))
                wg_sb.append(wgt)

            bg_sb = const.tile([P, NE], F32)
            bh_sb = const.tile([P, NE], F32)
            h0_sb = const.tile([P, NE], F32)
            nc.sync.dma_start(out=bg_sb, in_=bg_r)
            nc.sync.dma_start(out=bh_sb, in_=bh_r)
            nc.sync.dma_start(out=h0_sb, in_=h0_r)

            # ---- chunk 0, phase 1: kd-outer waves over 4 concurrent PSUM
            # banks; each wg[kd] DMA unblocks a whole wave level on arrival.
            gt0 = [None] * NE
            wave0_kd_mm = {}
            phase1_mid_mm = None
            for wave in range(2):
                es = list(range(wave * 4, wave * 4 + 4))
                pgs = {
                    e: pp.tile([P, NCH], F32, tag="pg", name=f"pg0_{e}")
                    for e in es
                }
                for kd in range(KD):
                    for e in es:
                        mm = nc.tensor.matmul(
                            pgs[e],
                            lhsT=wg_sb[kd][:, e * P : (e + 1) * P],
                            rhs=xin0[:, kd, :],
                            start=(kd == 0),
                            stop=(kd == KD - 1),
                        )
                    if wave == 0:
                        wave0_kd_mm[kd] = mm
                for e in es:
                    g = gc.tile([P, NCH], F32, tag=f"g{e}", name=f"g0_{e}")
                    nc.scalar.activation(
                        out=g, in_=pgs[e], func=ACT.Sigmoid,
                        bias=bg_sb[:, e : e + 1],
                    )
                    gt0[e] = g
                if wave == 0:
                    phase1_mid_mm = mm

            # Wh weights stream while phase 1 runs: wh[kd] waits on the
            # wave-0 matmul that consumed wg[kd].
            wh_sb = []
            for kd in range(KD):
                wht = const.tile([P, D], W_DT, tag=f"wh{kd}", name=f"wh_sb{kd}")
                dma = nc.sync.dma_start(out=wht, in_=whT_r[:, kd, :])
                tile.add_dep_helper(
                    dma.ins, wave0_kd_mm[kd].ins, sync=True, reason="pace wh behind wg"
                )
                wh_sb.append(wht)

            prev_h = [None] * NE

            def c_unit(n, e, gtile, xin, t0=0, tn=NCH):
                """c projection + pointwise + scan + store for tokens
                [t0, tn) of chunk n, output block e.  gtile holds the full
                chunk's g; the sub-range is sliced out of it."""
                w = tn - t0
                lsl = slice(n * NCH + t0, n * NCH + tn)
                tsl = slice(t0, tn)
                esl = slice(e * P, (e + 1) * P)
                pc = pp.tile([P, w], F32, tag="pc", name=f"pc_{n}_{e}_{t0}")
                for kd in range(KD):
                    nc.tensor.matmul(
                        pc,
                        lhsT=wh_sb[kd][:, esl],
                        rhs=xin[:, kd, tsl],
                        start=(kd == 0),
                        stop=(kd == KD - 1),
                    )
                c = gc.tile([P, w], F32, tag="c", name=f"c_{n}_{e}_{t0}")
                nc.scalar.activation(
                    out=c, in_=pc, func=ACT.Tanh, bias=bh_sb[:, e : e + 1]
                )
                d1 = gc.tile([P, w], F32, tag="d1", name=f"d1_{n}_{e}_{t0}")
                nc.vector.scalar_tensor_tensor(
                    out=d1, in0=gtile[:, tsl], scalar=1.0, in1=c,
                    op0=ALU.subtract, op1=ALU.mult,
                )
                if n == 0 and t0 == 0:
                    init = h0_sb[:, e : e + 1]
                else:
                    pw = prev_h[e].shape[-1]
                    init = prev_h[e][:, pw - 1 : pw]
                h = hpool.tile([P, w], F32, tag=f"h{e}", name=f"h_{n}_{e}_{t0}")
                nc.vector.tensor_tensor_scan(
                    out=h, data0=gtile[:, tsl], data1=d1, initial=init,
                    op0=ALU.mult, op1=ALU.subtract,
                )
                prev_h[e] = h
                nc.sync.dma_start(out=out_r[:, e, lsl], in_=h)

            # ---- chunk 0, phase 2
            for e in range(NE):
                c_unit(0, e, gt0[e], xin0)

            # ---- chunks 1..7: interleaved per-e units
            for n in range(1, NCHUNK):
                lsl = slice(n * NCH, (n + 1) * NCH)
                xin = xpool.tile([P, KD, NCH], MM_DT, tag="xin", name=f"xin_{n}")
                dma = nc.scalar.dma_start(out=xin, in_=xT_r[:, :, lsl])
                if n == 1:
                    # keep xin1 from competing with the startup weight stream
                    tile.add_dep_helper(
                        dma.ins, phase1_mid_mm.ins, sync=True, reason="pace xin1"
                    )
                for e in range(NE):
                    esl = slice(e * P, (e + 1) * P)
                    pg = pp.tile([P, NCH], F32, tag="pg", name=f"pg_{n}_{e}")
                    for kd in range(KD):
                        nc.tensor.matmul(
                            pg,
                            lhsT=wg_sb[kd][:, esl],
                            rhs=xin[:, kd, :],
                            start=(kd == 0),
                            stop=(kd == KD - 1),
                        )
                    g = gc.tile([P, NCH], F32, tag=f"g{e}", name=f"g_{n}_{e}")
                    nc.scalar.activation(
                        out=g, in_=pg, func=ACT.Sigmoid, bias=bg_sb[:, e : e + 1]
                    )
                    if n == NCHUNK - 1 and e == NE - 1:
                        # Final unit: halve it so the very last
                        # tanh+scan+store tail is half as long.
                        c_unit(n, e, g, xin, 0, NCH // 2)
                        c_unit(n, e, g, xin, NCH // 2, NCH)
                    else:
                        c_unit(n, e, g, xin)

    nc.compile()
    return nc


_NC_CACHE = None


def _get_nc():
    global _NC_CACHE
    if _NC_CACHE is None:
        _NC_CACHE = build_nc()
    return _NC_CACHE


def kernel(x, hidden, Wg, bg, Wh, bh):
    x = np.ascontiguousarray(np.asarray(x, dtype=np.float32))
    hidden = np.ascontiguousarray(np.asarray(hidden, dtype=np.float32))
    Wg = np.asarray(Wg, dtype=np.float32)
    bg = np.ascontiguousarray(np.asarray(bg, dtype=np.float32))
    Wh = np.asarray(Wh, dtype=np.float32)
    bh = np.ascontiguousarray(np.asarray(bh, dtype=np.float32))

    nc = _get_nc()

    xT = np.ascontiguousarray(x.transpose(0, 2, 1))   # [B, D, L]
    WgT = np.ascontiguousarray(Wg.T)
    WhT = np.ascontiguousarray(Wh.T)

    in_maps = [
        {
            "xT": xT[b],
            "WgT": WgT,
            "WhT": WhT,
            "bg": bg,
            "bh": bh,
            "hidden": hidden[b],
        }
        for b in range(B)
    ]
    res = bass_utils.run_bass_kernel_spmd(nc, in_maps, core_ids=list(range(B)))
    out = np.stack([res.results[b]["outT"].T for b in range(B)])  # [B, L, D]
    return np.ascontiguousarray(out.astype(np.float32))
